# revision 18
# baseline (speedup 1.0000x reference)
"""CrossViewLoss (SimCLR-style NT-Xent) on 8 trn2 NeuronCores.

Math: with z = row-normalized emb, only the two cross-view blocks of the
[2N,2N] similarity survive the mask, and both are views of the single
[N,N] matrix S = z_i @ z_j.T:
    loss * 2N = sum_m [log(0.5*rowsum_m) - 4*pos_m]
              + sum_c  log(0.5*colsum_c)
where rowsum/colsum are row/col sums of exp(2*S) and pos = diag(S).

Sharding: rows of S across 8 cores (512 anchor rows each).  Collectives
are deliberately avoided (they cost milliseconds on this runtime): every
core receives the full emb_j plus its own row slices, builds the
normalized z_j^T [256,4096] locally (norms -> scale -> PE transpose,
pipelined per 512-column group with the main loop), computes its row
block of exp(2*S) with fused row sums (activation accum_out) and
column-sum accumulation (ones-vector matmuls into PSUM), and returns a
[1, 4104] partial (4096 colsum partials + its scalar row-term sum).
The host sums the 8 partials and finishes the few thousand logs in
numpy — microseconds of f64 work.
"""

import numpy as np

N = 4096
D = 256
C = 8
SLICE = N // C          # 512 rows per core
P = 128
MI = SLICE // P         # 4 row tiles per core
KC = D // P             # 2 contraction chunks
NJT = 512               # n-chunk (PSUM bank) size
NJ = N // NJT           # 8 n-chunks / column groups
QJ = N // P             # 32 row tiles of full emb_j
OUT_LEN = N + 8         # colsum partials [4096] + P_c at [4096] + pad

_CACHE = {}


def _build_nc(reps=1):
    import concourse.mybir as mybir
    import concourse.tile as tile
    from concourse import bacc
    from concourse.masks import make_identity

    dt = mybir.dt
    f32 = dt.float32
    f32r = dt.float32r
    AF = mybir.ActivationFunctionType
    X = mybir.AxisListType.X

    nc = bacc.Bacc("TRN2", target_bir_lowering=False, debug=False, num_devices=C)

    emb_i_sl = nc.dram_tensor("emb_i_sl", [SLICE, D], f32, kind="ExternalInput")
    emb_j_sl = nc.dram_tensor("emb_j_sl", [SLICE, D], f32, kind="ExternalInput")
    emb_j_full = nc.dram_tensor("emb_j_full", [N, D], f32, kind="ExternalInput")
    out = nc.dram_tensor("out", [1, OUT_LEN], f32, kind="ExternalOutput")

    with tile.TileContext(nc) as tc:
        with (
            tc.tile_pool(name="persist", bufs=1) as persist,
            tc.tile_pool(name="scr", bufs=3) as scr,
            tc.tile_pool(name="exp", bufs=4) as expp,
            tc.tile_pool(name="ps_t", bufs=2, space="PSUM") as ps_t,
            tc.tile_pool(name="ps_g", bufs=3, space="PSUM") as ps_g,
            tc.tile_pool(name="ps_cs", bufs=2, space="PSUM") as ps_cs,
        ):

            def body():
                identity = persist.tile([P, P], f32, name="identity")
                make_identity(nc, identity[:])
                ones_f = persist.tile([P, 1], f32, name="ones_f")
                nc.gpsimd.memset(ones_f[:], 1.0)
                ones_r = persist.tile([P, 1], f32r, name="ones_r")
                nc.vector.tensor_copy(ones_r[:], ones_f[:])
                # dummy Ln pulls the natural_log_exp ACT table load off the
                # critical path (runs while the input DMAs are in flight)
                warm = persist.tile([P, 1], f32, name="warm")
                nc.scalar.activation(warm[:], ones_f[:], AF.Ln)

                # ---- own-slice loads (norms, lhsT, pos) ----
                nat_i = persist.tile([P, MI, D], f32, name="nat_i")
                nc.sync.dma_start(
                    nat_i[:], emb_i_sl[:].rearrange("(q p) d -> p q d", p=P)
                )
                nat_js = persist.tile([P, MI, D], f32, name="nat_js")
                nc.sync.dma_start(
                    nat_js[:], emb_j_sl[:].rearrange("(q p) d -> p q d", p=P)
                )
                # full emb_j, partition-tiled [128, 32, 256] (4 MB)
                nat_jf = persist.tile([P, QJ, D], f32, name="nat_jf")
                nc.sync.dma_start(
                    nat_jf[:], emb_j_full[:].rearrange("(q p) d -> p q d", p=P)
                )

                # row norms of the two own slices -> inv norms
                nsq_i = persist.tile([P, MI], f32, name="nsq_i")
                nsq_js = persist.tile([P, MI], f32, name="nsq_js")
                for q in range(MI):
                    sq = scr.tile([P, D], f32, name="sq")
                    nc.vector.tensor_mul(sq[:], nat_i[:, q, :], nat_i[:, q, :])
                    nc.vector.reduce_sum(nsq_i[:, q : q + 1], sq[:], axis=X)
                for q in range(MI):
                    sq = scr.tile([P, D], f32, name="sq")
                    nc.vector.tensor_mul(sq[:], nat_js[:, q, :], nat_js[:, q, :])
                    nc.vector.reduce_sum(nsq_js[:, q : q + 1], sq[:], axis=X)
                lnn_i = persist.tile([P, MI], f32, name="lnn_i")
                nc.scalar.activation(lnn_i[:], nsq_i[:], AF.Ln)
                invn_i = persist.tile([P, MI], f32, name="invn_i")
                nc.scalar.activation(invn_i[:], lnn_i[:], AF.Exp, scale=-0.5)
                lnn_js = persist.tile([P, MI], f32, name="lnn_js")
                nc.scalar.activation(lnn_js[:], nsq_js[:], AF.Ln)
                invn_js = persist.tile([P, MI], f32, name="invn_js")
                nc.scalar.activation(invn_js[:], lnn_js[:], AF.Exp, scale=-0.5)
                scale2 = persist.tile([P, MI], f32, name="scale2")
                nc.vector.tensor_scalar_mul(scale2[:], invn_i[:], 2.0)
                invij = persist.tile([P, MI], f32, name="invij")
                nc.vector.tensor_mul(invij[:], invn_i[:], invn_js[:])

                # raw emb_i^T (inv_ni folded into the exp scale later)
                lhsT = [
                    persist.tile([P, SLICE], f32r, name=f"lhsT{k}") for k in range(KC)
                ]
                for q in range(MI):
                    for k in range(KC):
                        pst = ps_t.tile([P, P], f32, name="pst")
                        nc.tensor.transpose(
                            pst[:], nat_i[:, q, k * P : (k + 1) * P], identity[:]
                        )
                        nc.vector.tensor_copy(lhsT[k][:, q * P : (q + 1) * P], pst[:])

                # pos = (emb_i . emb_j) * inv_ni * inv_nj, pre-scaled by 4
                rawdot = persist.tile([P, MI], f32, name="rawdot")
                for q in range(MI):
                    prod = scr.tile([P, D], f32, name="prod")
                    nc.vector.tensor_mul(prod[:], nat_i[:, q, :], nat_js[:, q, :])
                    nc.vector.reduce_sum(rawdot[:, q : q + 1], prod[:], axis=X)
                pos4 = persist.tile([P, MI], f32, name="pos4")
                nc.vector.tensor_mul(pos4[:], rawdot[:], invij[:])
                pos4m4 = persist.tile([P, MI], f32, name="pos4m4")
                nc.vector.tensor_scalar_mul(pos4m4[:], pos4[:], 4.0)

                # ---- main pipeline: per 512-column group, build that slice
                # of z_j^T (norms -> scale -> transpose), then its S-block
                # column chunk with fused exp/rowsum/colsum ----
                rhs = [persist.tile([P, N], f32r, name=f"rhs{k}") for k in range(KC)]
                cs_sb = persist.tile([1, OUT_LEN], f32, name="cs_sb")
                nc.gpsimd.memset(cs_sb[0:1, N + 1 : OUT_LEN], 0.0)
                rsparts = persist.tile([P, MI * NJ], f32, name="rsparts")

                for nj in range(NJ):
                    # prep column group nj from emb_j rows [nj*512,(nj+1)*512)
                    nsqg = scr.tile([P, MI], f32, name="nsqg", bufs=2)
                    for q in range(MI):
                        r = nj * MI + q
                        sq = scr.tile([P, D], f32, name="sqg")
                        nc.vector.tensor_mul(sq[:], nat_jf[:, r, :], nat_jf[:, r, :])
                        nc.vector.reduce_sum(nsqg[:, q : q + 1], sq[:], axis=X)
                    lng = scr.tile([P, MI], f32, name="lng", bufs=2)
                    nc.scalar.activation(lng[:], nsqg[:], AF.Ln)
                    invg = scr.tile([P, MI], f32, name="invg", bufs=2)
                    nc.scalar.activation(invg[:], lng[:], AF.Exp, scale=-0.5)
                    for q in range(MI):
                        r = nj * MI + q
                        zj = scr.tile([P, D], f32, name="zj")
                        nc.vector.tensor_scalar_mul(
                            zj[:], nat_jf[:, r, :], invg[:, q : q + 1]
                        )
                        for k in range(KC):
                            pst = ps_t.tile([P, P], f32, name="pst")
                            nc.tensor.transpose(
                                pst[:], zj[:, k * P : (k + 1) * P], identity[:]
                            )
                            nc.scalar.copy(
                                rhs[k][:, nj * NJT + q * P : nj * NJT + (q + 1) * P],
                                pst[:],
                            )

                    # S-block column chunk nj
                    cs_ps = ps_cs.tile([1, NJT], f32, name="cs_ps")
                    for mi in range(MI):
                        g = ps_g.tile([P, NJT], f32, name="g")
                        for k in range(KC):
                            nc.tensor.matmul(
                                g[:],
                                lhsT[k][:, mi * P : (mi + 1) * P],
                                rhs[k][:, nj * NJT : (nj + 1) * NJT],
                                start=(k == 0),
                                stop=(k == KC - 1),
                            )
                        e = expp.tile([P, NJT], f32r, name="e")
                        col = mi * NJ + nj
                        nc.scalar.activation(
                            e[:],
                            g[:],
                            AF.Exp,
                            scale=scale2[:, mi : mi + 1],
                            accum_out=rsparts[:, col : col + 1],
                        )
                        nc.tensor.matmul(
                            cs_ps[:],
                            ones_r[:],
                            e[:],
                            start=(mi == 0),
                            stop=(mi == MI - 1),
                            skip_group_check=True,
                        )
                    nc.vector.tensor_copy(
                        cs_sb[0:1, nj * NJT : (nj + 1) * NJT], cs_ps[:]
                    )

                # ---- per-core scalar P_c = sum(log(0.5*rowsum) - 4*pos) ----
                rs4 = persist.tile([P, MI], f32, name="rs4")
                nc.vector.reduce_sum(
                    rs4[:], rsparts[:].rearrange("p (m j) -> p m j", j=NJ), axis=X
                )
                lg4 = persist.tile([P, MI], f32, name="lg4")
                nc.scalar.activation(lg4[:], rs4[:], AF.Ln, scale=0.5)
                rowterm = persist.tile([P, MI], f32, name="rowterm")
                nc.vector.tensor_sub(rowterm[:], lg4[:], pos4m4[:])
                rowv = persist.tile([P, 1], f32, name="rowv")
                nc.vector.reduce_sum(rowv[:], rowterm[:], axis=X)
                p_ps = ps_t.tile([1, 1], f32, name="p_ps", tag="pst")
                nc.tensor.matmul(p_ps[:], rowv[:], ones_f[:])
                nc.scalar.copy(cs_sb[0:1, N : N + 1], p_ps[:])

                nc.sync.dma_start(out[:], cs_sb[:])

            for _rep in range(reps):
                body()

    nc.compile()
    return nc


def kernel(emb_i, emb_j):
    from concourse.bass_utils import run_bass_kernel_spmd

    if "nc" not in _CACHE:
        _CACHE["nc"] = _build_nc()
    nc = _CACHE["nc"]

    emb_i = np.ascontiguousarray(np.asarray(emb_i, dtype=np.float32))
    emb_j = np.ascontiguousarray(np.asarray(emb_j, dtype=np.float32))
    in_maps = [
        {
            "emb_i_sl": emb_i[c * SLICE : (c + 1) * SLICE],
            "emb_j_sl": emb_j[c * SLICE : (c + 1) * SLICE],
            "emb_j_full": emb_j,
        }
        for c in range(C)
    ]
    res = run_bass_kernel_spmd(nc, in_maps, list(range(C)))
    parts = np.stack(
        [np.asarray(res.results[c]["out"], dtype=np.float64)[0] for c in range(C)]
    )
    colsum = parts[:, :N].sum(axis=0)
    p_total = parts[:, N].sum()
    loss = (p_total + np.log(0.5 * colsum).sum()) / (2.0 * N)
    return np.float32(loss).reshape(())
